# revision 8
# baseline (speedup 1.0000x reference)
"""Trainium2 Bass kernel for DigitConvolutionalModel (dense_cnn).

Model: x[B,784] -> 3x3 valid conv on 28x28 -> flatten(676) -> fc1(128)+relu
       -> fc2(10).

Strategy:
  * The conv and fc1 are both linear, so fold the conv into fc1 on the host
    (weight prep, ~0.7 MFLOP): W_eff[128,784] with
    W_eff[o, 28r+c] = sum_{di,dj} conv_w[di,dj] * fc1_w[o, 26(r-di)+(c-dj)].
    Device computes logits = relu(x @ W_eff.T + b1) @ fc2_w.T + b2.
  * Pure data parallel: batch 65536 -> 8 cores x 8192 rows, weights replicated.
  * x is shipped per-core transposed [784, 8192] and cast to fp16 during the
    host-side shard marshalling: halves the HBM traffic (the kernel is
    memory-bound) while fp16's 10 mantissa bits + fp32 PSUM accumulation keep
    rel err ~3e-4. PE runs fp16 at full rate.
  * K=784 on partitions as 6x128 chunks + 16 tail; chunk-major matmul order
    with the K-tail matmul first, so each PSUM tile completes right after its
    last chunk DMA lands (minimal tail latency).
  * DGE ring separation: x-chunk DMAs alone on the SP HWDGE ring (no
    head-of-line blocking), weight loads on the ScalarE ring, out-DMAs on
    GpSimd SWDGE.
  * relu+bias fused in one ScalarE activation (PSUM->SBUF, fp16 out); fc2 is
    a tiny second matmul; fc2 bias-add on VectorE; output written transposed
    [10, 8192] per core and untransposed on the host.
  * Final groups tapered 2048->1024->512 to shrink the compute tail after the
    last DMA.
"""

import numpy as np

B_FULL = 65536
N_CORES = 8
B_LOC = B_FULL // N_CORES  # 8192
KF = 784  # input features (28*28)
P = 128  # SBUF partitions / K chunk
KCH = KF // P  # 6 full K chunks
KT = KF - KCH * P  # 16 tail rows
H = 128  # hidden
O = 10  # output classes
NT = 512  # PSUM tile width (fp32 bank limit)
G = 2048  # batch-group width

_CACHED = {}


def _taper_groups(b_loc, group):
    groups, pos, rem = [], 0, b_loc
    while rem > 2 * group:
        groups.append((pos, group))
        pos += group
        rem -= group
    while rem > 0:
        gw = max(512, min(group, rem // 2 if rem > 512 else 512))
        if rem <= 512:
            gw = rem
        groups.append((pos, gw))
        pos += gw
        rem -= gw
    return groups


def _build_nc(b_loc=B_LOC, group=G, xbufs=18, ps1_bufs=6, ps2_bufs=2, h_bufs=6,
              repeat=1):
    import concourse.bacc as bacc
    import concourse.tile as tile
    from concourse import mybir

    f32 = mybir.dt.float32
    fx = mybir.dt.float16
    AF = mybir.ActivationFunctionType

    nc = bacc.Bacc(
        "TRN2", target_bir_lowering=False, debug=False, enable_asserts=False,
        enable_partition_id=False,
    )
    xT = nc.dram_tensor("xT", [KF, b_loc], fx, kind="ExternalInput")
    w1t = nc.dram_tensor("w1t", [KF, H], fx, kind="ExternalInput")
    w2t = nc.dram_tensor("w2t", [H, O], fx, kind="ExternalInput")
    bias = nc.dram_tensor("bias", [H, 2], f32, kind="ExternalInput")
    outT = nc.dram_tensor("outT", [O, b_loc], f32, kind="ExternalOutput")

    groups = _taper_groups(b_loc, group)

    with tile.TileContext(nc) as tc:
        with (
            tc.tile_pool(name="wpool", bufs=1) as wpool,
            tc.tile_pool(name="xpool", bufs=xbufs) as xpool,
            tc.tile_pool(name="tailpool", bufs=1) as tailpool,
            tc.tile_pool(name="hpool", bufs=h_bufs) as hpool,
            tc.tile_pool(name="opool", bufs=h_bufs) as opool,
            tc.tile_pool(name="ps1", bufs=ps1_bufs, space="PSUM") as ps1,
            tc.tile_pool(name="ps2", bufs=ps2_bufs, space="PSUM") as ps2,
        ):
            # Weights/biases: few packed DMAs, on the ScalarE HWDGE ring so
            # the SP ring stays exclusive to x-chunk streaming.
            w1 = wpool.tile([P, KCH * H], fx)
            nc.scalar.dma_start(
                w1[:].rearrange("p (c m) -> p c m", c=KCH),
                w1t[0 : KCH * P, :].rearrange("(c p) m -> p c m", p=P),
            )
            w1tail = wpool.tile([KT, H], fx)
            nc.scalar.dma_start(w1tail[:], w1t[KCH * P : KF, :])
            w2 = wpool.tile([H, O], fx)
            nc.scalar.dma_start(w2[:], w2t[:])
            bs = wpool.tile([H, 2], f32)
            nc.scalar.dma_start(bs[:], bias[:])
            b1s = bs[:, 0:1]
            b2s = bs[0:O, 1:2]

            # K tail rows of x for the whole shard (one DMA, resident).
            xtail = tailpool.tile([KT, b_loc], fx)
            nc.scalar.dma_start(xtail[:], xT[KCH * P : KF, :])

            for rep in range(repeat):
                for n0, gw in groups:
                    xg = []
                    for c in range(KCH):
                        t = xpool.tile(
                            [P, gw],
                            fx,
                            name=f"xg{rep}_{n0}_{c}",
                            tag=f"xg{gw}",
                            bufs=(xbufs if gw == group else 2 * KCH),
                        )
                        nc.sync.dma_start(t[:], xT[c * P : (c + 1) * P, n0 : n0 + gw])
                        xg.append(t)
                    jts = gw // NT
                    psums = []
                    for j in range(jts):
                        psum = ps1.tile([P, NT], f32, name=f"ps_{n0}_{j}", tag="ps")
                        nc.tensor.matmul(
                            psum[:],
                            w1tail[:],
                            xtail[:, n0 + j * NT : n0 + (j + 1) * NT],
                            start=True,
                            stop=False,
                        )
                        psums.append(psum)
                    for c in range(KCH):
                        for j in range(jts):
                            nc.tensor.matmul(
                                psums[j][:],
                                w1[:, c * H : (c + 1) * H],
                                xg[c][:, j * NT : (j + 1) * NT],
                                start=False,
                                stop=(c == KCH - 1),
                            )
                    for j in range(jts):
                        nj = n0 + j * NT
                        h = hpool.tile([P, NT], fx, name=f"h_{nj}", tag="h")
                        nc.scalar.activation(
                            h[:], psums[j][:], AF.Relu, bias=b1s[:], scale=1.0
                        )
                        psum2 = ps2.tile([O, NT], f32, name=f"ps2_{nj}", tag="ps2")
                        nc.tensor.matmul(psum2[:], w2[:], h[:], start=True, stop=True)
                        o = opool.tile([O, NT], f32, name=f"o_{nj}", tag="o")
                        nc.vector.tensor_scalar_add(o[:], psum2[:], b2s[:])
                        nc.gpsimd.dma_start(outT[:, nj : nj + NT], o[:])
    nc.compile()
    return nc


def _get_nc():
    if "nc" not in _CACHED:
        _CACHED["nc"] = _build_nc()
    return _CACHED["nc"]


def _prep_weights(conv_w, fc1_w, fc1_b, fc2_w, fc2_b):
    w1 = np.asarray(fc1_w, np.float64).reshape(H, 26, 26)
    cw = np.asarray(conv_w, np.float64)
    w_eff = np.zeros((H, 28, 28), np.float64)
    for di in range(3):
        for dj in range(3):
            w_eff[:, di : di + 26, dj : dj + 26] += cw[di, dj] * w1
    w1t = np.ascontiguousarray(w_eff.reshape(H, KF).T.astype(np.float16))  # [784,128]
    w2t = np.ascontiguousarray(np.asarray(fc2_w, np.float32).T.astype(np.float16))
    bias = np.zeros((H, 2), np.float32)
    bias[:, 0] = np.asarray(fc1_b, np.float32)
    bias[:O, 1] = np.asarray(fc2_b, np.float32)
    return w1t, w2t, bias


def kernel(x, conv_w, fc1_w, fc1_b, fc2_w, fc2_b, _trace=False):
    from concourse.bass_utils import run_bass_kernel_spmd

    x = np.asarray(x, np.float32)
    assert x.shape == (B_FULL, KF), x.shape
    w1t, w2t, bias = _prep_weights(conv_w, fc1_w, fc1_b, fc2_w, fc2_b)

    x16 = x.astype(np.float16)
    in_maps = []
    for c in range(N_CORES):
        xT = np.ascontiguousarray(x16[c * B_LOC : (c + 1) * B_LOC].T)  # [784, 8192]
        in_maps.append({"xT": xT, "w1t": w1t, "w2t": w2t, "bias": bias})

    nc = _get_nc()
    try:
        res = run_bass_kernel_spmd(
            nc, in_maps, core_ids=list(range(N_CORES)), trace=_trace
        )
    except ModuleNotFoundError:
        # axon NTFF profile hook not present in this container; run untraced
        import os

        os.environ["BASS_NEVER_TRACE"] = "1"
        res = run_bass_kernel_spmd(
            nc, in_maps, core_ids=list(range(N_CORES)), trace=False
        )
    out = np.empty((B_FULL, O), np.float32)
    for c in range(N_CORES):
        out[c * B_LOC : (c + 1) * B_LOC] = res.results[c]["outT"].T
    if _trace:
        _CACHED["last_results"] = res
    return out


# revision 9
# speedup vs baseline: 1.1465x; 1.1465x over previous
"""Trainium2 Bass kernel for DigitConvolutionalModel (dense_cnn).

Model: x[B,784] -> 3x3 valid conv on 28x28 -> flatten(676) -> fc1(128)+relu
       -> fc2(10).

Strategy:
  * The conv and fc1 are both linear, so fold the conv into fc1 on the host
    (weight prep, ~0.7 MFLOP): W_eff[128,784] with
    W_eff[o, 28r+c] = sum_{di,dj} conv_w[di,dj] * fc1_w[o, 26(r-di)+(c-dj)].
    Device computes logits = relu(x @ W_eff.T + b1) @ fc2_w.T + b2.
  * Pure data parallel: batch 65536 -> 8 cores x 8192 rows, weights replicated.
  * x is shipped per-core transposed [784, 8192] and cast to fp16 during the
    host-side shard marshalling: halves the HBM traffic (the kernel is
    memory-bound) while fp16's 10 mantissa bits + fp32 PSUM accumulation keep
    rel err ~3e-4. PE runs fp16 at full rate.
  * K=784 on partitions as 6x128 chunks + 16 tail; chunk-major matmul order
    with the K-tail matmul first, so each PSUM tile completes right after its
    last chunk DMA lands (minimal tail latency).
  * DGE ring separation: x-chunk DMAs alone on the SP HWDGE ring (no
    head-of-line blocking), weight loads on the ScalarE ring, out-DMAs on
    GpSimd SWDGE.
  * relu+bias fused in one ScalarE activation (PSUM->SBUF, fp16 out); fc2 is
    a tiny second matmul; fc2 bias-add on VectorE; output written transposed
    [10, 8192] per core and untransposed on the host.
  * Final groups tapered 2048->1024->512 to shrink the compute tail after the
    last DMA.
"""

import numpy as np

B_FULL = 65536
N_CORES = 8
B_LOC = B_FULL // N_CORES  # 8192
KF = 784  # input features (28*28)
P = 128  # SBUF partitions / K chunk
KCH = KF // P  # 6 full K chunks
KT = KF - KCH * P  # 16 tail rows
H = 128  # hidden
O = 10  # output classes
NT = 512  # PSUM tile width (fp32 bank limit)
G = 2048  # batch-group width

_CACHED = {}


def _taper_groups(b_loc, group):
    groups, pos, rem = [], 0, b_loc
    while rem > 2 * group:
        groups.append((pos, group))
        pos += group
        rem -= group
    while rem > 0:
        gw = max(512, min(group, rem // 2 if rem > 512 else 512))
        if rem <= 512:
            gw = rem
        groups.append((pos, gw))
        pos += gw
        rem -= gw
    return groups


def _build_nc(b_loc=B_LOC, group=G, xbufs=18, ps1_bufs=6, ps2_bufs=2, h_bufs=6,
              repeat=1):
    import concourse.bacc as bacc
    import concourse.tile as tile
    from concourse import mybir

    f32 = mybir.dt.float32
    fx = mybir.dt.float16
    AF = mybir.ActivationFunctionType

    nc = bacc.Bacc(
        "TRN2", target_bir_lowering=False, debug=False, enable_asserts=False,
        enable_partition_id=False,
    )
    xT = nc.dram_tensor("xT", [KF, b_loc], fx, kind="ExternalInput")
    w1t = nc.dram_tensor("w1t", [KF, H], fx, kind="ExternalInput")
    w2t = nc.dram_tensor("w2t", [H, O], fx, kind="ExternalInput")
    bias = nc.dram_tensor("bias", [H, 2], f32, kind="ExternalInput")
    outT = nc.dram_tensor("outT", [O, b_loc], f32, kind="ExternalOutput")

    groups = _taper_groups(b_loc, group)

    with tile.TileContext(nc) as tc:
        with (
            tc.tile_pool(name="wpool", bufs=1) as wpool,
            tc.tile_pool(name="xpool", bufs=xbufs) as xpool,
            tc.tile_pool(name="tailpool", bufs=1) as tailpool,
            tc.tile_pool(name="hpool", bufs=h_bufs) as hpool,
            tc.tile_pool(name="opool", bufs=h_bufs) as opool,
            tc.tile_pool(name="ps1", bufs=ps1_bufs, space="PSUM") as ps1,
            tc.tile_pool(name="ps2", bufs=ps2_bufs, space="PSUM") as ps2,
        ):
            # Weights/biases: few packed DMAs, on the ScalarE HWDGE ring so
            # the SP ring stays exclusive to x-chunk streaming. w1tail+xtail
            # go first: every PSUM group's K-tail matmul needs them.
            w1tail = wpool.tile([KT, H], fx)
            nc.scalar.dma_start(w1tail[:], w1t[KCH * P : KF, :])
            xtail = tailpool.tile([KT, b_loc], fx)
            nc.scalar.dma_start(xtail[:], xT[KCH * P : KF, :])
            w1 = wpool.tile([P, KCH * H], fx)
            nc.scalar.dma_start(
                w1[:].rearrange("p (c m) -> p c m", c=KCH),
                w1t[0 : KCH * P, :].rearrange("(c p) m -> p c m", p=P),
            )
            w2 = wpool.tile([H, O], fx)
            nc.scalar.dma_start(w2[:], w2t[:])
            bs = wpool.tile([H, 2], f32)
            nc.scalar.dma_start(bs[:], bias[:])
            b1s = bs[:, 0:1]
            b2s = bs[0:O, 1:2]

            for rep in range(repeat):
                for n0, gw in groups:
                    xg = []
                    for c in range(KCH):
                        t = xpool.tile(
                            [P, gw],
                            fx,
                            name=f"xg{rep}_{n0}_{c}",
                            tag=f"xg{gw}",
                            bufs=(xbufs if gw == group else 2 * KCH),
                        )
                        nc.sync.dma_start(t[:], xT[c * P : (c + 1) * P, n0 : n0 + gw])
                        xg.append(t)
                    jts = gw // NT
                    psums = []
                    for j in range(jts):
                        psum = ps1.tile([P, NT], f32, name=f"ps_{n0}_{j}", tag="ps")
                        nc.tensor.matmul(
                            psum[:],
                            w1tail[:],
                            xtail[:, n0 + j * NT : n0 + (j + 1) * NT],
                            start=True,
                            stop=False,
                        )
                        psums.append(psum)
                    for c in range(KCH):
                        for j in range(jts):
                            nc.tensor.matmul(
                                psums[j][:],
                                w1[:, c * H : (c + 1) * H],
                                xg[c][:, j * NT : (j + 1) * NT],
                                start=False,
                                stop=(c == KCH - 1),
                            )
                    for j in range(jts):
                        nj = n0 + j * NT
                        h = hpool.tile([P, NT], fx, name=f"h_{nj}", tag="h")
                        nc.scalar.activation(
                            h[:], psums[j][:], AF.Relu, bias=b1s[:], scale=1.0
                        )
                        psum2 = ps2.tile([O, NT], f32, name=f"ps2_{nj}", tag="ps2")
                        nc.tensor.matmul(psum2[:], w2[:], h[:], start=True, stop=True)
                        o = opool.tile([O, NT], f32, name=f"o_{nj}", tag="o")
                        nc.vector.tensor_scalar_add(o[:], psum2[:], b2s[:])
                        nc.gpsimd.dma_start(outT[:, nj : nj + NT], o[:])
    nc.compile()
    return nc


def _get_nc():
    if "nc" not in _CACHED:
        _CACHED["nc"] = _build_nc()
    return _CACHED["nc"]


def _prep_weights(conv_w, fc1_w, fc1_b, fc2_w, fc2_b):
    w1 = np.asarray(fc1_w, np.float64).reshape(H, 26, 26)
    cw = np.asarray(conv_w, np.float64)
    w_eff = np.zeros((H, 28, 28), np.float64)
    for di in range(3):
        for dj in range(3):
            w_eff[:, di : di + 26, dj : dj + 26] += cw[di, dj] * w1
    w1t = np.ascontiguousarray(w_eff.reshape(H, KF).T.astype(np.float16))  # [784,128]
    w2t = np.ascontiguousarray(np.asarray(fc2_w, np.float32).T.astype(np.float16))
    bias = np.zeros((H, 2), np.float32)
    bias[:, 0] = np.asarray(fc1_b, np.float32)
    bias[:O, 1] = np.asarray(fc2_b, np.float32)
    return w1t, w2t, bias


def kernel(x, conv_w, fc1_w, fc1_b, fc2_w, fc2_b, _trace=False):
    from concourse.bass_utils import run_bass_kernel_spmd

    x = np.asarray(x, np.float32)
    assert x.shape == (B_FULL, KF), x.shape
    w1t, w2t, bias = _prep_weights(conv_w, fc1_w, fc1_b, fc2_w, fc2_b)

    x16 = x.astype(np.float16)
    in_maps = []
    for c in range(N_CORES):
        xT = np.ascontiguousarray(x16[c * B_LOC : (c + 1) * B_LOC].T)  # [784, 8192]
        in_maps.append({"xT": xT, "w1t": w1t, "w2t": w2t, "bias": bias})

    nc = _get_nc()
    try:
        res = run_bass_kernel_spmd(
            nc, in_maps, core_ids=list(range(N_CORES)), trace=_trace
        )
    except ModuleNotFoundError:
        # axon NTFF profile hook not present in this container; run untraced
        import os

        os.environ["BASS_NEVER_TRACE"] = "1"
        res = run_bass_kernel_spmd(
            nc, in_maps, core_ids=list(range(N_CORES)), trace=False
        )
    out = np.empty((B_FULL, O), np.float32)
    for c in range(N_CORES):
        out[c * B_LOC : (c + 1) * B_LOC] = res.results[c]["outT"].T
    if _trace:
        _CACHED["last_results"] = res
    return out
